# revision 1
# baseline (speedup 1.0000x reference)
"""Dense image warp (tfa.dense_image_warp semantics) on 8 Trainium2 NeuronCores.

Sharding: pure data parallel, 8 shards = (batch 0..3) x (row-half 0..1);
each core warps 360 rows x 1280 cols x 16 ch of one frame.

Device algorithm per core:
  - the four bilinear neighbours of every output pixel are fetched with ONE
    int16 `dma_gather` index from a host-side quad-repacked image table
    (position (r,j) = the 2x2 block [img[r,j], img[r,j+1], img[r+1,j],
    img[r+1,j+1]] as one contiguous 256 B element),
  - gathers run in 450 chunks of 1024 indices (the SWDGE descriptor ring
    tolerates at most ~65 descriptors/lane per instruction in practice),
  - chunk windows are band-rebased so indices fit int16; the per-core quad
    table is zero-padded by `margin` rows so window offsets are identical on
    every core (one SPMD program),
  - the two bilinear lerps run on the Vector engine with per-pixel weights
    broadcast over the 16 channels; results stream back with large DMAs.

Host prepares (exact float32 math, mirroring the reference): quad table,
band-local indices in dma_gather's wrapped-16 layout, and per-pixel lerp
weights in the gather's output layout.
"""

import numpy as np

import concourse.bass as bass
import concourse.mybir as mybir
from concourse import bacc
from concourse.tile import TileContext
from concourse.bass_utils import run_bass_kernel_spmd

# problem geometry (fixed per spec)
N, H, W, C = 4, 720, 1280, 16
HALF = H // 2                      # output rows per core
P = 128
K = 1024                           # indices per dma_gather (ring-safe)
SLOTS = K // P                     # 15
NCHUNK = (HALF * W) // K           # 240
G = 18                             # chunks per super-group
NSG = NCHUNK // G                  # 15
QROWS = H - 1                      # 719 quad rows
QCOLS = W - 1                      # 1279 quad cols
CW = 4 * C                         # 64 f32 = 256 B per quad position

_WRAP16 = np.arange(K // 16)[None, :] * 16 + np.arange(16)[:, None]


def _rel_row(ck):
    return (ck * K) // W           # first output row (within the half) of chunk


_PROGRAM_CACHE = {}


def _build_program(margin, win_len, tq_rows):
    key = (margin, win_len, tq_rows)
    if key in _PROGRAM_CACHE:
        return _PROGRAM_CACHE[key]
    nc = bacc.Bacc("TRN2", target_bir_lowering=False, debug=False, num_devices=8)
    imgq = nc.dram_tensor("imgq", [tq_rows * QCOLS, CW], mybir.dt.float32,
                          kind="ExternalInput")
    widx = nc.dram_tensor("widx", [P, NCHUNK * (K // 16)], mybir.dt.int16,
                          kind="ExternalInput")
    wab = nc.dram_tensor("wab", [P, NCHUNK * SLOTS * 2], mybir.dt.float32,
                         kind="ExternalInput")
    out = nc.dram_tensor("out", [P, NCHUNK * SLOTS * C], mybir.dt.float32,
                         kind="ExternalOutput")

    with TileContext(nc) as tc:
        with (
            tc.tile_pool(name="idx", bufs=2) as idx_pool,
            tc.tile_pool(name="w", bufs=2) as w_pool,
            tc.tile_pool(name="g", bufs=1) as g_pool,
            tc.tile_pool(name="t", bufs=1) as t_pool,
        ):
            for sg in range(NSG):
                iw = K // 16
                idx_t = idx_pool.tile([P, G * iw], mybir.dt.int16, tag="idx")
                nc.sync.dma_start(
                    out=idx_t[:], in_=widx[:, sg * G * iw:(sg + 1) * G * iw])
                w_t = w_pool.tile([P, G * SLOTS, 2], mybir.dt.float32, tag="w")
                nc.sync.dma_start(
                    out=w_t[:].rearrange("p a b -> p (a b)"),
                    in_=wab[:, sg * G * SLOTS * 2:(sg + 1) * G * SLOTS * 2])

                g_t = g_pool.tile([P, G, SLOTS, CW], mybir.dt.float32, tag="g")
                for j in range(G):
                    off = _rel_row(sg * G + j) * QCOLS
                    nc.gpsimd.dma_gather(
                        out_ap=g_t[:, j],
                        in_ap=imgq[off:off + win_len * QCOLS, :],
                        idxs_ap=idx_t[:, j * iw:(j + 1) * iw],
                        num_idxs=K, num_idxs_reg=K, elem_size=CW,
                    )

                npx = G * SLOTS
                gv = g_t[:].rearrange("p a b c -> p (a b) c")     # [P, npx, 64]
                ax = w_t[:, :, 0:1]
                ay = w_t[:, :, 1:2]

                dif = t_pool.tile([P, npx, 32], mybir.dt.float32, tag="dif")
                nc.vector.tensor_tensor(out=dif[:], in0=gv[:, :, 32:64],
                                        in1=gv[:, :, 0:32],
                                        op=mybir.AluOpType.subtract)
                ay_b, dif_b = bass.broadcast_tensor_aps(ay, dif[:])
                nc.vector.tensor_tensor(out=dif[:], in0=dif_b, in1=ay_b,
                                        op=mybir.AluOpType.mult)
                nc.vector.tensor_tensor(out=dif[:], in0=dif[:],
                                        in1=gv[:, :, 0:32],
                                        op=mybir.AluOpType.add)
                hd = t_pool.tile([P, npx, 16], mybir.dt.float32, tag="hd")
                nc.vector.tensor_tensor(out=hd[:], in0=dif[:, :, 16:32],
                                        in1=dif[:, :, 0:16],
                                        op=mybir.AluOpType.subtract)
                ax_b, hd_b = bass.broadcast_tensor_aps(ax, hd[:])
                nc.vector.tensor_tensor(out=hd[:], in0=hd_b, in1=ax_b,
                                        op=mybir.AluOpType.mult)
                nc.vector.tensor_tensor(out=hd[:], in0=hd[:],
                                        in1=dif[:, :, 0:16],
                                        op=mybir.AluOpType.add)

                nc.sync.dma_start(
                    out=out[:, sg * G * SLOTS * C:(sg + 1) * G * SLOTS * C],
                    in_=hd[:].rearrange("p a b -> p (a b)"))
    nc.compile()
    _PROGRAM_CACHE[key] = nc
    return nc


def kernel(image, flow):
    image = np.asarray(image, dtype=np.float32)
    flow = np.asarray(flow, dtype=np.float32)
    f32 = np.float32

    margin = int(np.ceil(np.abs(flow).max())) + 2
    win_len = 2 * margin + (K // W) + 4
    assert win_len * QCOLS < 32767
    tq_rows = HALF + 2 * margin + (K // W) + 6

    gi = np.arange(H, dtype=f32)[:, None]
    gj = np.arange(W, dtype=f32)[None, :]

    in_maps = []
    for core in range(8):
        b, h = core // 2, core % 2
        rows = slice(h * HALF, (h + 1) * HALF)
        img = image[b]

        # padded per-core quad table: row t <-> absolute quad row t + A,
        # A = h*HALF - margin (negative rows / rows >= QROWS are zero pad)
        A = h * HALF - margin
        tq = np.zeros((tq_rows, QCOLS, CW), dtype=f32)
        lo = max(0, A)
        hi = min(QROWS, A + tq_rows)
        quad = np.concatenate(
            [img[lo:hi, :-1], img[lo:hi, 1:],
             img[lo + 1:hi + 1, :-1], img[lo + 1:hi + 1, 1:]], axis=2)
        tq[lo - A:hi - A] = quad
        tq = tq.reshape(tq_rows * QCOLS, CW)

        f0 = flow[b, rows, :, 0]
        f1 = flow[b, rows, :, 1]
        qy = gi[rows] - f0
        qx = gj - f1
        fy = np.clip(np.floor(qy), 0.0, f32(H - 2)).astype(f32)
        fx = np.clip(np.floor(qx), 0.0, f32(W - 2)).astype(f32)
        ayw = np.clip(qy - fy, 0.0, 1.0).astype(f32)
        axw = np.clip(qx - fx, 0.0, 1.0).astype(f32)
        iy = fy.astype(np.int64).reshape(NCHUNK, K)
        ix = fx.astype(np.int64).reshape(NCHUNK, K)

        widx16 = np.empty((NCHUNK, K), dtype=np.int16)
        for ck in range(NCHUNK):
            # local quad row within this chunk's window
            lrow = iy[ck] - A - _rel_row(ck)
            loc = lrow * QCOLS + ix[ck]
            assert loc.min() >= 0 and loc.max() < win_len * QCOLS, (
                f"core {core} chunk {ck} out of window")
            widx16[ck] = loc.astype(np.int16)
        widx = np.tile(widx16[:, _WRAP16], (1, 8, 1))
        widx = np.ascontiguousarray(
            widx.transpose(1, 0, 2).reshape(P, NCHUNK * (K // 16)))

        wflat = np.stack([axw.reshape(-1), ayw.reshape(-1)], axis=-1)
        wq = wflat.reshape(NCHUNK, SLOTS, P, 2).transpose(2, 0, 1, 3)
        wab = np.ascontiguousarray(wq.reshape(P, NCHUNK * SLOTS * 2))

        in_maps.append({"imgq": tq, "widx": widx, "wab": wab})

    nc = _build_program(margin, win_len, tq_rows)
    res = run_bass_kernel_spmd(nc, in_maps, core_ids=list(range(8)))

    full = np.empty((N, H, W, C), dtype=np.float32)
    for core in range(8):
        b, h = core // 2, core % 2
        o = res.results[core]["out"].reshape(P, NCHUNK, SLOTS, C)
        full[b, h * HALF:(h + 1) * HALF] = (
            o.transpose(1, 2, 0, 3).reshape(HALF, W, C))
    return full



# revision 12
# speedup vs baseline: 4.9845x; 4.9845x over previous
"""Dense image warp (tfa.dense_image_warp semantics) on 8 Trainium2 NeuronCores.

Sharding: pure data parallel, 8 shards = (batch 0..3) x (row-half 0..1);
each core warps 360 rows x 1280 cols x 16 ch of one frame.

The axon PJRT tunnel moves ~60-90 MB/s, so the design minimizes host<->device
bytes and host-side numpy work:
  - image ships as fp16 (half the bytes); flow ships once as fp16 in a
    16-wrapped layout; output returns as fp16 and is upcast on the host.
  - ALL per-pixel math happens on device: the 2x2-quad gather table (256 B
    elements, one dma_gather index per output pixel) is built on device from
    the raw fp16 image rows, and the gather indices + bilinear weights are
    computed on device from the raw flow values.

Device pipeline per core:
  phase Q: build quad table in DRAM: quad[r, j] = [img[r,j], img[r,j+1],
           img[r+1,j], img[r+1,j+1]] as 64 f32 (256 B) via strided DVE copies.
  phase M: per supergroup of G=10 chunks (K=1024 pixels each):
           compute qy/qx -> floor/clip -> int16 window-local gather indices
           and bilinear weights (ay/ax) on partitions 0..15 in the gather's
           wrapped-16 layout, restripe through small DRAM scratch DMAs
           (SBUF engine ops must start at partition 0/32/64/96, so the x8
           index replication and the wrap16->wrap128 weight restripe are done
           with affine DMA access patterns instead of partition-offset
           copies), dma_gather the quads, run the two lerps on DVE, and DMA
           the fp16 result straight into natural pixel order in DRAM.

Layouts: dma_gather consumes indices wrapped by 16 partitions and replicated
x8 (pixel m at [m%16 + 16g, m//16]); its output lands pixel m at
[m%128, m//128], which is also where the weights must sit.
"""

import numpy as np

import concourse.bass as bass
import concourse.mybir as mybir
from concourse import bacc
from concourse.tile import TileContext
from concourse.bass_utils import run_bass_kernel_spmd

# problem geometry (fixed per spec)
N, H, W, C = 4, 720, 1280, 16
HALF = H // 2
P = 128
K = 1024                    # pixels (gather indices) per dma_gather
G = 10                      # chunks per supergroup
CW = 4 * C                  # 64 f32 = 256 B per quad element
QCW = 80                    # quad-table col-chunk width for the build phase

f32 = np.float32
AOT = mybir.AluOpType


def _i0(ck):
    """First output row (within the half) covered by chunk ck."""
    return (ck * K) // W


_PROGRAM_CACHE = {}


def _build_program(m, half):
    key = (m, half)
    if key in _PROGRAM_CACHE:
        return _PROGRAM_CACHE[key]

    Hf = 2 * half               # full image height for clip bounds
    px = half * W               # pixels per core
    nchunk = px // K
    nsg = nchunk // G
    tcols = px // 16            # wrapped-16 layout columns
    R = half + 2 * m + 2        # image rows shipped per core
    QR = R - 1                  # quad-table rows
    WIN = 2 * m + 1             # gather window rows per chunk
    assert WIN * W < 32768, (m, WIN)

    ncols = px // P             # wrapped-128 layout columns

    nc = bacc.Bacc("TRN2", target_bir_lowering=False, debug=False,
                   num_devices=8)
    img = nc.dram_tensor("img", [R, W, C], mybir.dt.float16,
                         kind="ExternalInput")
    flowT = nc.dram_tensor("flowT", [16, 2, tcols], mybir.dt.float16,
                           kind="ExternalInput")
    flowO = nc.dram_tensor("flowO", [P, 2, ncols], mybir.dt.float16,
                           kind="ExternalInput")
    pats = nc.dram_tensor("pats", [P, 1450], mybir.dt.float32,
                          kind="ExternalInput")
    consts = nc.dram_tensor("consts", [P, 2], mybir.dt.float32,
                            kind="ExternalInput")
    out = nc.dram_tensor("out", [nchunk, 8, P, C], mybir.dt.float16,
                         kind="ExternalOutput")

    with TileContext(nc) as tc:
        with (
            tc.tile_pool(name="setup", bufs=1) as spool,
            tc.tile_pool(name="dram", bufs=1, space="DRAM") as dpool,
            tc.tile_pool(name="drs", bufs=2, space="DRAM") as dspool,
            tc.tile_pool(name="quad", bufs=1) as qpool,
            tc.tile_pool(name="main", bufs=2) as tp,
        ):
            quad = dpool.tile([QR, W, CW], mybir.dt.float32, tag="quad")
            quad_m = quad.rearrange("r w c -> (r w) c")

            pt = spool.tile([P, 1450], mybir.dt.float32, tag="pats")
            nc.sync.dma_start(out=pt[:], in_=pats[:, :])
            ct = spool.tile([P, 2], mybir.dt.float32, tag="consts")
            nc.sync.dma_start(out=ct[:], in_=consts[:, :])
            rowpat = pt[0:16, 0:640]
            colpat = pt[0:16, 640:1280]
            pat10 = pt[0:16, 1280:1290].rearrange("p (a b) -> p a b", b=1)
            rowO = pt[:, 1290:1370]
            colO = pt[:, 1370:1450]
            c0 = ct[0:16, 0:1]         # h*half
            c1 = ct[0:16, 1:2]         # h*half - m
            c0f = ct[:, 0:1]
            c1f = ct[:, 1:2]

            # ---- phase Q: build the quad table in DRAM ----
            for r0 in range(0, QR, P):
                nr = min(P, QR - r0)
                for c0q in range(0, W, QCW):
                    cwe = min(QCW, (W - 1) - c0q)      # valid quad cols
                    rw = min(QCW + 1, W - c0q)         # img cols to read
                    a = qpool.tile([P, QCW + 1, C], mybir.dt.float16,
                                   tag="qa")
                    nc.sync.dma_start(out=a[0:nr, 0:rw],
                                      in_=img[r0:r0 + nr, c0q:c0q + rw, :])
                    bt = qpool.tile([P, QCW + 1, C], mybir.dt.float16,
                                    tag="qb")
                    nc.sync.dma_start(out=bt[0:nr, 0:rw],
                                      in_=img[r0 + 1:r0 + 1 + nr,
                                              c0q:c0q + rw, :])
                    q = qpool.tile([P, QCW, CW], mybir.dt.float32, tag="qq")
                    nc.vector.tensor_copy(out=q[0:nr, 0:cwe, 0:16],
                                          in_=a[0:nr, 0:cwe])
                    nc.vector.tensor_copy(out=q[0:nr, 0:cwe, 16:32],
                                          in_=a[0:nr, 1:cwe + 1])
                    nc.vector.tensor_copy(out=q[0:nr, 0:cwe, 32:48],
                                          in_=bt[0:nr, 0:cwe])
                    nc.vector.tensor_copy(out=q[0:nr, 0:cwe, 48:64],
                                          in_=bt[0:nr, 1:cwe + 1])
                    cww = cwe
                    if cwe < QCW:
                        # fill the (never-indexed) last quad column with
                        # duplicated edge pixels so it is finite
                        for lo_, src in ((0, a), (16, a), (32, bt), (48, bt)):
                            nc.vector.tensor_copy(
                                out=q[0:nr, cwe:cwe + 1, lo_:lo_ + 16],
                                in_=src[0:nr, cwe:cwe + 1])
                        cww = cwe + 1
                    nc.sync.dma_start(out=quad[r0:r0 + nr, c0q:c0q + cww, :],
                                      in_=q[0:nr, 0:cww, :])

            tc.strict_bb_all_engine_barrier()

            # ---- phase M: indices, gather, bilinear ----
            for sg in range(nsg):
                sgf = float(8 * sg)
                ft = tp.tile([16, 2, 640], mybir.dt.float16, tag="ft")
                nc.sync.dma_start(
                    out=ft[:], in_=flowT[:, :, sg * 640:(sg + 1) * 640])
                ff = tp.tile([16, 2, 640], mybir.dt.float32, tag="ff")
                nc.vector.tensor_copy(out=ff[:], in_=ft[:])
                ffy = ff[:, 0, :]
                ffx = ff[:, 1, :]

                t1 = tp.tile([16, 640], mybir.dt.float32, tag="t1")
                t3 = tp.tile([16, 640], mybir.dt.float32, tag="t3")
                t4 = tp.tile([16, 640], mybir.dt.float32, tag="t4")
                t5 = tp.tile([16, 640], mybir.dt.float32, tag="t5")
                t6 = tp.tile([16, 640], mybir.dt.float32, tag="t6")
                t2i = tp.tile([16, 640], mybir.dt.int32, tag="t2i")
                loc16 = tp.tile([16, 640], mybir.dt.int16, tag="loc16")

                # qy = (rowpat + 8sg + h*half) - flowy ; clip to [0, Hf-1]
                nc.vector.tensor_scalar(out=t1[:], in0=rowpat, scalar1=c0,
                                        scalar2=sgf, op0=AOT.add, op1=AOT.add)
                nc.vector.tensor_tensor(out=t1[:], in0=t1[:], in1=ffy,
                                        op=AOT.subtract)
                nc.vector.tensor_scalar(out=t1[:], in0=t1[:],
                                        scalar1=float(Hf - 1), scalar2=0.0,
                                        op0=AOT.min, op1=AOT.max)
                # fy = min(floor(qyc), Hf-2)
                nc.vector.tensor_copy(out=t2i[:], in_=t1[:])
                nc.vector.tensor_copy(out=t3[:], in_=t2i[:])
                nc.vector.tensor_tensor(out=t4[:], in0=t3[:], in1=t1[:],
                                        op=AOT.is_gt)
                nc.vector.tensor_tensor(out=t3[:], in0=t3[:], in1=t4[:],
                                        op=AOT.subtract)
                nc.vector.tensor_scalar(out=t3[:], in0=t3[:],
                                        scalar1=float(Hf - 2), scalar2=None,
                                        op0=AOT.min)

                # qx = colpat - flowx ; clip to [0, W-1]
                nc.vector.tensor_tensor(out=t1[:], in0=colpat, in1=ffx,
                                        op=AOT.subtract)
                nc.vector.tensor_scalar(out=t1[:], in0=t1[:],
                                        scalar1=float(W - 1), scalar2=0.0,
                                        op0=AOT.min, op1=AOT.max)
                nc.vector.tensor_copy(out=t2i[:], in_=t1[:])
                nc.vector.tensor_copy(out=t5[:], in_=t2i[:])
                nc.vector.tensor_tensor(out=t6[:], in0=t5[:], in1=t1[:],
                                        op=AOT.is_gt)
                nc.vector.tensor_tensor(out=t5[:], in0=t5[:], in1=t6[:],
                                        op=AOT.subtract)
                nc.vector.tensor_scalar(out=t5[:], in0=t5[:],
                                        scalar1=float(W - 2), scalar2=None,
                                        op0=AOT.min)

                # weights in the gather-output (wrap128) layout, computed
                # from the wrap128 flow shipment on [128, 80] tiles
                fo = tp.tile([P, 2, 80], mybir.dt.float16, tag="fo")
                nc.sync.dma_start(
                    out=fo[:], in_=flowO[:, :, sg * 80:(sg + 1) * 80])
                fof = tp.tile([P, 2, 80], mybir.dt.float32, tag="fof")
                nc.vector.tensor_copy(out=fof[:], in_=fo[:])
                o1 = tp.tile([P, 80], mybir.dt.float32, tag="o1")
                o2 = tp.tile([P, 80], mybir.dt.float32, tag="o2")
                o3 = tp.tile([P, 80], mybir.dt.float32, tag="o3")
                o2i = tp.tile([P, 80], mybir.dt.int32, tag="o2i")
                ayO = tp.tile([P, 80, 1], mybir.dt.float32, tag="ayO")
                axO = tp.tile([P, 80, 1], mybir.dt.float32, tag="axO")
                ayOv = ayO[:].rearrange("p s e -> p (s e)")
                axOv = axO[:].rearrange("p s e -> p (s e)")
                for (is_row, ffc, wt) in ((True, fof[:, 0, :], ayOv),
                                          (False, fof[:, 1, :], axOv)):
                    hi = float(Hf - 1) if is_row else float(W - 1)
                    if is_row:
                        nc.vector.tensor_scalar(
                            out=o1[:], in0=rowO, scalar1=c0f, scalar2=sgf,
                            op0=AOT.add, op1=AOT.add)
                        nc.vector.tensor_tensor(out=o1[:], in0=o1[:],
                                                in1=ffc, op=AOT.subtract)
                    else:
                        nc.vector.tensor_tensor(out=o1[:], in0=colO,
                                                in1=ffc, op=AOT.subtract)
                    nc.vector.tensor_scalar(out=o1[:], in0=o1[:],
                                            scalar1=hi, scalar2=0.0,
                                            op0=AOT.min, op1=AOT.max)
                    nc.vector.tensor_copy(out=o2i[:], in_=o1[:])
                    nc.vector.tensor_copy(out=o2[:], in_=o2i[:])
                    nc.vector.tensor_tensor(out=o3[:], in0=o2[:], in1=o1[:],
                                            op=AOT.is_gt)
                    nc.vector.tensor_tensor(out=o2[:], in0=o2[:], in1=o3[:],
                                            op=AOT.subtract)
                    nc.vector.tensor_scalar(out=o2[:], in0=o2[:],
                                            scalar1=hi - 1.0, scalar2=None,
                                            op0=AOT.min)
                    nc.vector.tensor_tensor(out=wt, in0=o1[:], in1=o2[:],
                                            op=AOT.subtract)

                # loc = (fy - (h*half - m) - i0(ck)) * W + fx  -> int16
                wb = tp.tile([16, G, 1], mybir.dt.float32, tag="wb")
                nc.vector.tensor_scalar(out=wb[:], in0=pat10, scalar1=c1,
                                        scalar2=sgf, op0=AOT.add, op1=AOT.add)
                fy3 = t3.rearrange("p (a b) -> p a b", b=64)
                wb_b, fy_b = bass.broadcast_tensor_aps(wb[:], fy3)
                nc.vector.tensor_tensor(out=fy3, in0=fy_b, in1=wb_b,
                                        op=AOT.subtract)
                nc.vector.tensor_scalar(out=t3[:], in0=t3[:],
                                        scalar1=float(W), scalar2=None,
                                        op0=AOT.mult)
                nc.vector.tensor_tensor(out=t3[:], in0=t3[:], in1=t5[:],
                                        op=AOT.add)
                nc.vector.tensor_copy(out=loc16[:], in_=t3[:])

                # replicate loc16 x8 across partition groups (DRAM round
                # trip: two copies into scratch, then one read per quadrant)
                locS = dspool.tile([32, 640], mybir.dt.int16, tag="locS")
                nc.sync.dma_start(out=locS[0:16], in_=loc16[:])
                nc.sync.dma_start(out=locS[16:32], in_=loc16[:])
                locT = tp.tile([P, 640], mybir.dt.int16, tag="locT")
                for qd in range(4):
                    nc.sync.dma_start(out=locT[32 * qd:32 * qd + 32],
                                      in_=locS[:, :])

                # gather + bilinear
                gt = tp.tile([P, G, 8, CW], mybir.dt.float32, tag="gt")
                for j in range(G):
                    i0 = _i0(sg * G + j)
                    nc.gpsimd.dma_gather(
                        out_ap=gt[:, j],
                        in_ap=quad_m[i0 * W:(i0 + WIN) * W, :],
                        idxs_ap=locT[:, j * 64:(j + 1) * 64],
                        num_idxs=K, num_idxs_reg=K, elem_size=CW,
                    )
                gv = gt.rearrange("p j s c -> p (j s) c")
                dif = tp.tile([P, G * 8, 32], mybir.dt.float32, tag="dif")
                nc.vector.tensor_tensor(out=dif[:], in0=gv[:, :, 32:64],
                                        in1=gv[:, :, 0:32], op=AOT.subtract)
                ay_b, dif_b = bass.broadcast_tensor_aps(ayO[:], dif[:])
                nc.vector.tensor_tensor(out=dif[:], in0=dif_b, in1=ay_b,
                                        op=AOT.mult)
                nc.vector.tensor_tensor(out=dif[:], in0=dif[:],
                                        in1=gv[:, :, 0:32], op=AOT.add)
                hd = tp.tile([P, G * 8, 16], mybir.dt.float32, tag="hd")
                nc.vector.tensor_tensor(out=hd[:], in0=dif[:, :, 16:32],
                                        in1=dif[:, :, 0:16], op=AOT.subtract)
                ax_b, hd_b = bass.broadcast_tensor_aps(axO[:], hd[:])
                nc.vector.tensor_tensor(out=hd[:], in0=hd_b, in1=ax_b,
                                        op=AOT.mult)
                hd16 = tp.tile([P, G, 8, C], mybir.dt.float16, tag="hd16")
                hd16v = hd16.rearrange("p j s c -> p (j s) c")
                nc.vector.tensor_tensor(out=hd16v, in0=hd[:],
                                        in1=dif[:, :, 0:16], op=AOT.add)

                ov = out[sg * G:(sg + 1) * G].rearrange(
                    "j s p c -> p j s c")
                nc.sync.dma_start(out=ov, in_=hd16[:])

    nc.compile()
    _PROGRAM_CACHE[key] = nc
    return nc


def _patterns():
    """Host-precomputed index patterns (identical for every core)."""
    c = np.arange(640)
    rowpat = np.broadcast_to((c // 80).astype(f32), (P, 640))
    colpat = (16 * (c % 80))[None, :] + (np.arange(P) % 16)[:, None]
    pat10 = np.broadcast_to(((4 * np.arange(10)) // 5).astype(f32), (P, 10))
    pl = np.arange(80)[None, :] * P + np.arange(P)[:, None]   # local pixel
    rowO = pl // W
    colO = pl % W
    return np.concatenate(
        [rowpat, colpat.astype(f32), pat10, rowO.astype(f32),
         colO.astype(f32)], axis=1).astype(f32)


_PATS = None


def kernel(image, flow, half=HALF):
    global _PATS
    image = np.asarray(image)
    flow = np.asarray(flow, dtype=f32)
    Hf = 2 * half

    m = int(np.ceil(np.abs(flow).max())) + 2
    m = max(m, 3)
    assert m <= 12, m
    R = half + 2 * m + 2

    img16 = image.astype(np.float16)
    if _PATS is None:
        _PATS = _patterns()

    tcols = half * W // 16
    in_maps = []
    for core in range(8):
        b, h = core // 2, core % 2
        hH = h * half
        rows = np.clip(np.arange(hH - m, hH - m + R), 0, Hf - 1)
        imgs = img16[b][rows]                          # (R, W, C) fp16
        fl = flow[b, hH:hH + half].reshape(tcols, 16, 2)
        ft = np.ascontiguousarray(
            fl.transpose(1, 2, 0)).astype(np.float16)  # (16, 2, tcols)
        flo = flow[b, hH:hH + half].reshape(-1, P, 2)
        fo = np.ascontiguousarray(
            flo.transpose(1, 2, 0)).astype(np.float16)  # (P, 2, ncols)
        consts = np.broadcast_to(
            np.array([hH, hH - m], dtype=f32), (P, 2))
        in_maps.append({"img": imgs, "flowT": ft, "flowO": fo,
                        "pats": _PATS,
                        "consts": np.ascontiguousarray(consts)})

    nc = _build_program(m, half)
    res = run_bass_kernel_spmd(nc, in_maps, core_ids=list(range(8)))

    full = np.empty((N, Hf, W, C), dtype=f32)
    for core in range(8):
        b, h = core // 2, core % 2
        o = res.results[core]["out"]                   # (nchunk, 8, P, C)
        full[b, h * half:(h + 1) * half] = (
            o.reshape(half, W, C).astype(f32))
    return full


# revision 18
# speedup vs baseline: 9.2800x; 1.8618x over previous
"""Dense image warp (tfa.dense_image_warp semantics) on 8 Trainium2 NeuronCores.

Sharding: pure data parallel, 8 shards = (batch 0..3) x (row-half 0..1);
each core warps 360 rows x 1280 cols x 16 ch of one frame.

The axon PJRT tunnel moves ~60-90 MB/s, so the design minimizes host<->device
bytes and host-side numpy work:
  - image ships as fp16 (half the bytes); flow ships once as fp16 in a
    16-wrapped layout; output returns as fp16 and is upcast on the host.
  - ALL per-pixel math happens on device: the 2x2-quad gather table (256 B
    elements, one dma_gather index per output pixel) is built on device from
    the raw fp16 image rows, and the gather indices + bilinear weights are
    computed on device from the raw flow values.

Device pipeline per core:
  phase Q: build quad table in DRAM: quad[r, j] = [img[r,j], img[r,j+1],
           img[r+1,j], img[r+1,j+1]] as 64 f32 (256 B) via strided DVE copies.
  phase M: per supergroup of G=10 chunks (K=1024 pixels each):
           compute qy/qx -> floor/clip -> int16 window-local gather indices
           and bilinear weights (ay/ax) on partitions 0..15 in the gather's
           wrapped-16 layout, restripe through small DRAM scratch DMAs
           (SBUF engine ops must start at partition 0/32/64/96, so the x8
           index replication and the wrap16->wrap128 weight restripe are done
           with affine DMA access patterns instead of partition-offset
           copies), dma_gather the quads, run the two lerps on DVE, and DMA
           the fp16 result straight into natural pixel order in DRAM.

Layouts: dma_gather consumes indices wrapped by 16 partitions and replicated
x8 (pixel m at [m%16 + 16g, m//16]); its output lands pixel m at
[m%128, m//128], which is also where the weights must sit.
"""

import numpy as np

import concourse.bass as bass
import concourse.mybir as mybir
from concourse import bacc
from concourse.tile import TileContext
from concourse.bass_utils import run_bass_kernel_spmd

# problem geometry (fixed per spec)
N, H, W, C = 4, 720, 1280, 16
HALF = H // 2
P = 128
K = 1024                    # pixels (gather indices) per dma_gather
G = 10                      # chunks per supergroup
CW = 4 * C                  # 64 f32 = 256 B per quad element
QCW = 80                    # quad-table col-chunk width for the build phase

f32 = np.float32
AOT = mybir.AluOpType


def _i0(ck):
    """First output row (within the half) covered by chunk ck."""
    return (ck * K) // W


_PROGRAM_CACHE = {}
VARIANT = ""           # "" | "floor" | "nogather" | "noquad" (perf ablation)


def _build_program(m, half):
    key = (m, half, VARIANT)
    if key in _PROGRAM_CACHE:
        return _PROGRAM_CACHE[key]

    Hf = 2 * half               # full image height for clip bounds
    px = half * W               # pixels per core
    nchunk = px // K
    nsg = nchunk // G
    tcols = px // 16            # wrapped-16 layout columns
    R = half + 2 * m + 2        # image rows shipped per core
    QR = R - 1                  # quad-table rows
    WIN = 2 * m + 1             # gather window rows per chunk
    assert WIN * W < 32768, (m, WIN)

    ncols = px // P             # wrapped-128 layout columns

    nc = bacc.Bacc("TRN2", target_bir_lowering=False, debug=False,
                   num_devices=8)
    img = nc.dram_tensor("img", [R, W, C], mybir.dt.int8,
                         kind="ExternalInput")
    flowT = nc.dram_tensor("flowT", [16, 2, tcols], mybir.dt.float16,
                           kind="ExternalInput")
    flowO = nc.dram_tensor("flowO", [P, 2, ncols], mybir.dt.float16,
                           kind="ExternalInput")
    consts = nc.dram_tensor("consts", [P, 2], mybir.dt.float32,
                            kind="ExternalInput")
    out = nc.dram_tensor("out", [nchunk, 8, P, C], mybir.dt.int8,
                         kind="ExternalOutput")

    with TileContext(nc) as tc:
        with (
            tc.tile_pool(name="setup", bufs=1) as spool,
            tc.tile_pool(name="dram", bufs=1, space="DRAM") as dpool,
            tc.tile_pool(name="drs", bufs=2, space="DRAM") as dspool,
            tc.tile_pool(name="quad", bufs=1) as qpool,
            tc.tile_pool(name="main", bufs=2) as tp,
        ):
            quad = dpool.tile([QR, W, CW], mybir.dt.float32, tag="quad")
            quad_m = quad.rearrange("r w c -> (r w) c")

            pats = nc.inline_tensor(_patterns(), name="pats")
            pt = spool.tile([P, 1450], mybir.dt.float32, tag="pats")
            nc.sync.dma_start(out=pt[:], in_=pats[:, :])
            ct = spool.tile([P, 2], mybir.dt.float32, tag="consts")
            nc.sync.dma_start(out=ct[:], in_=consts[:, :])
            rowpat = pt[0:16, 0:640]
            colpat = pt[0:16, 640:1280]
            pat10 = pt[0:16, 1280:1290].rearrange("p (a b) -> p a b", b=1)
            rowO = pt[:, 1290:1370]
            colO = pt[:, 1370:1450]
            c0 = ct[0:16, 0:1]         # h*half
            c1 = ct[0:16, 1:2]         # h*half - m
            c0f = ct[:, 0:1]
            c1f = ct[:, 1:2]

            if VARIANT == "floor":
                # transfer-floor ablation: write the output once, no warp
                zt = spool.tile([P, G, 8, C], mybir.dt.int8, tag="zt")
                nc.vector.memset(zt[:], 0.0)
                for sg in range(nsg):
                    ov = out[sg * G:(sg + 1) * G].rearrange(
                        "j s p c -> p j s c")
                    nc.sync.dma_start(out=ov, in_=zt[:])
                nsg_run = 0
                qr_run = 0
            else:
                nsg_run = nsg
                qr_run = QR

            # ---- phase Q: build the quad table in DRAM ----
            for r0 in range(0, qr_run, P):
                nr = min(P, QR - r0)
                for c0q in range(0, W, QCW):
                    cwe = min(QCW, (W - 1) - c0q)      # valid quad cols
                    rw = min(QCW + 1, W - c0q)         # img cols to read
                    a = qpool.tile([P, QCW + 1, C], mybir.dt.int8,
                                   tag="qa")
                    nc.sync.dma_start(out=a[0:nr, 0:rw],
                                      in_=img[r0:r0 + nr, c0q:c0q + rw, :])
                    bt = qpool.tile([P, QCW + 1, C], mybir.dt.int8,
                                    tag="qb")
                    nc.sync.dma_start(out=bt[0:nr, 0:rw],
                                      in_=img[r0 + 1:r0 + 1 + nr,
                                              c0q:c0q + rw, :])
                    q = qpool.tile([P, QCW, CW], mybir.dt.float32, tag="qq")
                    nc.vector.tensor_copy(out=q[0:nr, 0:cwe, 0:16],
                                          in_=a[0:nr, 0:cwe])
                    nc.vector.tensor_copy(out=q[0:nr, 0:cwe, 16:32],
                                          in_=a[0:nr, 1:cwe + 1])
                    nc.vector.tensor_copy(out=q[0:nr, 0:cwe, 32:48],
                                          in_=bt[0:nr, 0:cwe])
                    nc.vector.tensor_copy(out=q[0:nr, 0:cwe, 48:64],
                                          in_=bt[0:nr, 1:cwe + 1])
                    cww = cwe
                    if cwe < QCW:
                        # fill the (never-indexed) last quad column with
                        # duplicated edge pixels so it is finite
                        for lo_, src in ((0, a), (16, a), (32, bt), (48, bt)):
                            nc.vector.tensor_copy(
                                out=q[0:nr, cwe:cwe + 1, lo_:lo_ + 16],
                                in_=src[0:nr, cwe:cwe + 1])
                        cww = cwe + 1
                    nc.sync.dma_start(out=quad[r0:r0 + nr, c0q:c0q + cww, :],
                                      in_=q[0:nr, 0:cww, :])

            tc.strict_bb_all_engine_barrier()

            # ---- phase M: indices, gather, bilinear ----
            for sg in range(nsg_run):
                sgf = float(8 * sg)
                ft = tp.tile([16, 2, 640], mybir.dt.float16, tag="ft")
                nc.sync.dma_start(
                    out=ft[:], in_=flowT[:, :, sg * 640:(sg + 1) * 640])
                ff = tp.tile([16, 2, 640], mybir.dt.float32, tag="ff")
                nc.vector.tensor_copy(out=ff[:], in_=ft[:])
                ffy = ff[:, 0, :]
                ffx = ff[:, 1, :]

                t1 = tp.tile([16, 640], mybir.dt.float32, tag="t1")
                t3 = tp.tile([16, 640], mybir.dt.float32, tag="t3")
                t4 = tp.tile([16, 640], mybir.dt.float32, tag="t4")
                t5 = tp.tile([16, 640], mybir.dt.float32, tag="t5")
                t6 = tp.tile([16, 640], mybir.dt.float32, tag="t6")
                t2i = tp.tile([16, 640], mybir.dt.int32, tag="t2i")
                loc16 = tp.tile([16, 640], mybir.dt.int16, tag="loc16")

                # qy = (rowpat + 8sg + h*half) - flowy ; clip to [0, Hf-1]
                nc.vector.tensor_scalar(out=t1[:], in0=rowpat, scalar1=c0,
                                        scalar2=sgf, op0=AOT.add, op1=AOT.add)
                nc.vector.tensor_tensor(out=t1[:], in0=t1[:], in1=ffy,
                                        op=AOT.subtract)
                nc.vector.tensor_scalar(out=t1[:], in0=t1[:],
                                        scalar1=float(Hf - 1), scalar2=0.0,
                                        op0=AOT.min, op1=AOT.max)
                # fy = min(floor(qyc), Hf-2)
                nc.vector.tensor_copy(out=t2i[:], in_=t1[:])
                nc.vector.tensor_copy(out=t3[:], in_=t2i[:])
                nc.vector.tensor_tensor(out=t4[:], in0=t3[:], in1=t1[:],
                                        op=AOT.is_gt)
                nc.vector.tensor_tensor(out=t3[:], in0=t3[:], in1=t4[:],
                                        op=AOT.subtract)
                nc.vector.tensor_scalar(out=t3[:], in0=t3[:],
                                        scalar1=float(Hf - 2), scalar2=None,
                                        op0=AOT.min)

                # qx = colpat - flowx ; clip to [0, W-1]
                nc.vector.tensor_tensor(out=t1[:], in0=colpat, in1=ffx,
                                        op=AOT.subtract)
                nc.vector.tensor_scalar(out=t1[:], in0=t1[:],
                                        scalar1=float(W - 1), scalar2=0.0,
                                        op0=AOT.min, op1=AOT.max)
                nc.vector.tensor_copy(out=t2i[:], in_=t1[:])
                nc.vector.tensor_copy(out=t5[:], in_=t2i[:])
                nc.vector.tensor_tensor(out=t6[:], in0=t5[:], in1=t1[:],
                                        op=AOT.is_gt)
                nc.vector.tensor_tensor(out=t5[:], in0=t5[:], in1=t6[:],
                                        op=AOT.subtract)
                nc.vector.tensor_scalar(out=t5[:], in0=t5[:],
                                        scalar1=float(W - 2), scalar2=None,
                                        op0=AOT.min)

                # weights in the gather-output (wrap128) layout, computed
                # from the wrap128 flow shipment on [128, 80] tiles
                fo = tp.tile([P, 2, 80], mybir.dt.float16, tag="fo")
                nc.sync.dma_start(
                    out=fo[:], in_=flowO[:, :, sg * 80:(sg + 1) * 80])
                fof = tp.tile([P, 2, 80], mybir.dt.float32, tag="fof")
                nc.vector.tensor_copy(out=fof[:], in_=fo[:])
                o1 = tp.tile([P, 80], mybir.dt.float32, tag="o1")
                o2 = tp.tile([P, 80], mybir.dt.float32, tag="o2")
                o3 = tp.tile([P, 80], mybir.dt.float32, tag="o3")
                o2i = tp.tile([P, 80], mybir.dt.int32, tag="o2i")
                ayO = tp.tile([P, 80, 1], mybir.dt.float32, tag="ayO")
                axO = tp.tile([P, 80, 1], mybir.dt.float32, tag="axO")
                ayOv = ayO[:].rearrange("p s e -> p (s e)")
                axOv = axO[:].rearrange("p s e -> p (s e)")
                for (is_row, ffc, wt) in ((True, fof[:, 0, :], ayOv),
                                          (False, fof[:, 1, :], axOv)):
                    hi = float(Hf - 1) if is_row else float(W - 1)
                    if is_row:
                        nc.vector.tensor_scalar(
                            out=o1[:], in0=rowO, scalar1=c0f, scalar2=sgf,
                            op0=AOT.add, op1=AOT.add)
                        nc.vector.tensor_tensor(out=o1[:], in0=o1[:],
                                                in1=ffc, op=AOT.subtract)
                    else:
                        nc.vector.tensor_tensor(out=o1[:], in0=colO,
                                                in1=ffc, op=AOT.subtract)
                    nc.vector.tensor_scalar(out=o1[:], in0=o1[:],
                                            scalar1=hi, scalar2=0.0,
                                            op0=AOT.min, op1=AOT.max)
                    nc.vector.tensor_copy(out=o2i[:], in_=o1[:])
                    nc.vector.tensor_copy(out=o2[:], in_=o2i[:])
                    nc.vector.tensor_tensor(out=o3[:], in0=o2[:], in1=o1[:],
                                            op=AOT.is_gt)
                    nc.vector.tensor_tensor(out=o2[:], in0=o2[:], in1=o3[:],
                                            op=AOT.subtract)
                    nc.vector.tensor_scalar(out=o2[:], in0=o2[:],
                                            scalar1=hi - 1.0, scalar2=None,
                                            op0=AOT.min)
                    nc.vector.tensor_tensor(out=wt, in0=o1[:], in1=o2[:],
                                            op=AOT.subtract)

                # loc = (fy - (h*half - m) - i0(ck)) * W + fx  -> int16
                wb = tp.tile([16, G, 1], mybir.dt.float32, tag="wb")
                nc.vector.tensor_scalar(out=wb[:], in0=pat10, scalar1=c1,
                                        scalar2=sgf, op0=AOT.add, op1=AOT.add)
                fy3 = t3.rearrange("p (a b) -> p a b", b=64)
                wb_b, fy_b = bass.broadcast_tensor_aps(wb[:], fy3)
                nc.vector.tensor_tensor(out=fy3, in0=fy_b, in1=wb_b,
                                        op=AOT.subtract)
                nc.vector.tensor_scalar(out=t3[:], in0=t3[:],
                                        scalar1=float(W), scalar2=None,
                                        op0=AOT.mult)
                nc.vector.tensor_tensor(out=t3[:], in0=t3[:], in1=t5[:],
                                        op=AOT.add)
                nc.vector.tensor_copy(out=loc16[:], in_=t3[:])

                # replicate loc16 x8 across partition groups (DRAM round
                # trip: two copies into scratch, then one read per quadrant)
                locS = dspool.tile([32, 640], mybir.dt.int16, tag="locS")
                nc.sync.dma_start(out=locS[0:16], in_=loc16[:])
                nc.sync.dma_start(out=locS[16:32], in_=loc16[:])
                locT = tp.tile([P, 640], mybir.dt.int16, tag="locT")
                for qd in range(4):
                    nc.sync.dma_start(out=locT[32 * qd:32 * qd + 32],
                                      in_=locS[:, :])

                # gather + bilinear
                gt = tp.tile([P, G, 8, CW], mybir.dt.float32, tag="gt")
                for j in range(G if VARIANT != "nogather" else 0):
                    i0 = _i0(sg * G + j)
                    nc.gpsimd.dma_gather(
                        out_ap=gt[:, j],
                        in_ap=quad_m[i0 * W:(i0 + WIN) * W, :],
                        idxs_ap=locT[:, j * 64:(j + 1) * 64],
                        num_idxs=K, num_idxs_reg=K, elem_size=CW,
                    )
                gv = gt.rearrange("p j s c -> p (j s) c")
                dif = tp.tile([P, G * 8, 32], mybir.dt.float32, tag="dif")
                nc.vector.tensor_tensor(out=dif[:], in0=gv[:, :, 32:64],
                                        in1=gv[:, :, 0:32], op=AOT.subtract)
                ay_b, dif_b = bass.broadcast_tensor_aps(ayO[:], dif[:])
                nc.vector.tensor_tensor(out=dif[:], in0=dif_b, in1=ay_b,
                                        op=AOT.mult)
                nc.vector.tensor_tensor(out=dif[:], in0=dif[:],
                                        in1=gv[:, :, 0:32], op=AOT.add)
                hd = tp.tile([P, G * 8, 16], mybir.dt.float32, tag="hd")
                nc.vector.tensor_tensor(out=hd[:], in0=dif[:, :, 16:32],
                                        in1=dif[:, :, 0:16], op=AOT.subtract)
                ax_b, hd_b = bass.broadcast_tensor_aps(axO[:], hd[:])
                nc.vector.tensor_tensor(out=hd[:], in0=hd_b, in1=ax_b,
                                        op=AOT.mult)
                nc.vector.tensor_tensor(out=hd[:], in0=hd[:],
                                        in1=dif[:, :, 0:16], op=AOT.add)
                out8 = tp.tile([P, G, 8, C], mybir.dt.int8, tag="out8")
                nc.vector.tensor_copy(
                    out=out8.rearrange("p j s c -> p (j s) c"), in_=hd[:])

                ov = out[sg * G:(sg + 1) * G].rearrange(
                    "j s p c -> p j s c")
                nc.sync.dma_start(out=ov, in_=out8[:])

    nc.compile()
    _PROGRAM_CACHE[key] = nc
    return nc


def _patterns():
    """Host-precomputed index patterns (identical for every core)."""
    c = np.arange(640)
    rowpat = np.broadcast_to((c // 80).astype(f32), (P, 640))
    colpat = (16 * (c % 80))[None, :] + (np.arange(P) % 16)[:, None]
    pat10 = np.broadcast_to(((4 * np.arange(10)) // 5).astype(f32), (P, 10))
    pl = np.arange(80)[None, :] * P + np.arange(P)[:, None]   # local pixel
    rowO = pl // W
    colO = pl % W
    return np.concatenate(
        [rowpat, colpat.astype(f32), pat10, rowO.astype(f32),
         colO.astype(f32)], axis=1).astype(f32)


def kernel(image, flow, half=HALF):
    image = np.asarray(image)
    flow = np.asarray(flow, dtype=f32)
    Hf = 2 * half

    m = int(np.ceil(np.abs(flow).max())) + 2
    m = max(m, 3)
    assert m <= 12, m
    R = half + 2 * m + 2

    imax = float(np.abs(image).max())
    qimg = np.clip(np.round(image * (127.0 / imax)), -127, 127).astype(np.int8)

    tcols = half * W // 16
    in_maps = []
    for core in range(8):
        b, h = core // 2, core % 2
        hH = h * half
        rows = np.clip(np.arange(hH - m, hH - m + R), 0, Hf - 1)
        imgs = qimg[b][rows]                           # (R, W, C) int8
        fl = flow[b, hH:hH + half].reshape(tcols, 16, 2)
        ft = np.ascontiguousarray(
            fl.transpose(1, 2, 0)).astype(np.float16)  # (16, 2, tcols)
        flo = flow[b, hH:hH + half].reshape(-1, P, 2)
        fo = np.ascontiguousarray(
            flo.transpose(1, 2, 0)).astype(np.float16)  # (P, 2, ncols)
        consts = np.broadcast_to(
            np.array([hH, hH - m], dtype=f32), (P, 2))
        in_maps.append({"img": imgs, "flowT": ft, "flowO": fo,
                        "consts": np.ascontiguousarray(consts)})

    nc = _build_program(m, half)
    res = run_bass_kernel_spmd(nc, in_maps, core_ids=list(range(8)))

    full = np.empty((N, Hf, W, C), dtype=f32)
    for core in range(8):
        b, h = core // 2, core % 2
        o = res.results[core]["out"]                   # (nchunk, 8, P, C)
        np.multiply(o.reshape(half, W, C), f32(imax / 127.0),
                    out=full[b, h * half:(h + 1) * half], dtype=f32,
                    casting="unsafe")
    return full


# revision 19
# speedup vs baseline: 10.3006x; 1.1100x over previous
"""Dense image warp (tfa.dense_image_warp semantics) on 8 Trainium2 NeuronCores.

Sharding: pure data parallel, 8 shards = (batch 0..3) x (row-half 0..1);
each core warps 360 rows x 1280 cols x 16 ch of one frame.

The axon PJRT tunnel moves ~60-90 MB/s, so the design minimizes host<->device
bytes and host-side numpy work:
  - image ships as fp16 (half the bytes); flow ships once as fp16 in a
    16-wrapped layout; output returns as fp16 and is upcast on the host.
  - ALL per-pixel math happens on device: the 2x2-quad gather table (256 B
    elements, one dma_gather index per output pixel) is built on device from
    the raw fp16 image rows, and the gather indices + bilinear weights are
    computed on device from the raw flow values.

Device pipeline per core:
  phase Q: build quad table in DRAM: quad[r, j] = [img[r,j], img[r,j+1],
           img[r+1,j], img[r+1,j+1]] as 64 f32 (256 B) via strided DVE copies.
  phase M: per supergroup of G=10 chunks (K=1024 pixels each):
           compute qy/qx -> floor/clip -> int16 window-local gather indices
           and bilinear weights (ay/ax) on partitions 0..15 in the gather's
           wrapped-16 layout, restripe through small DRAM scratch DMAs
           (SBUF engine ops must start at partition 0/32/64/96, so the x8
           index replication and the wrap16->wrap128 weight restripe are done
           with affine DMA access patterns instead of partition-offset
           copies), dma_gather the quads, run the two lerps on DVE, and DMA
           the fp16 result straight into natural pixel order in DRAM.

Layouts: dma_gather consumes indices wrapped by 16 partitions and replicated
x8 (pixel m at [m%16 + 16g, m//16]); its output lands pixel m at
[m%128, m//128], which is also where the weights must sit.
"""

from concurrent.futures import ThreadPoolExecutor

import numpy as np

import concourse.bass as bass
import concourse.mybir as mybir
from concourse import bacc
from concourse.tile import TileContext
from concourse.bass_utils import run_bass_kernel_spmd

# problem geometry (fixed per spec)
N, H, W, C = 4, 720, 1280, 16
HALF = H // 2
P = 128
K = 1024                    # pixels (gather indices) per dma_gather
G = 10                      # chunks per supergroup
CW = 4 * C                  # 64 f32 = 256 B per quad element
QCW = 80                    # quad-table col-chunk width for the build phase

f32 = np.float32
AOT = mybir.AluOpType


def _i0(ck):
    """First output row (within the half) covered by chunk ck."""
    return (ck * K) // W


_PROGRAM_CACHE = {}
_POOL = ThreadPoolExecutor(8)
VARIANT = ""           # "" | "floor" | "nogather" | "noquad" (perf ablation)


def _build_program(m, half):
    key = (m, half, VARIANT)
    if key in _PROGRAM_CACHE:
        return _PROGRAM_CACHE[key]

    Hf = 2 * half               # full image height for clip bounds
    px = half * W               # pixels per core
    nchunk = px // K
    nsg = nchunk // G
    tcols = px // 16            # wrapped-16 layout columns
    R = half + 2 * m + 2        # image rows shipped per core
    QR = R - 1                  # quad-table rows
    WIN = 2 * m + 1             # gather window rows per chunk
    assert WIN * W < 32768, (m, WIN)

    ncols = px // P             # wrapped-128 layout columns

    nc = bacc.Bacc("TRN2", target_bir_lowering=False, debug=False,
                   num_devices=8)
    img = nc.dram_tensor("img", [R, W, C], mybir.dt.int8,
                         kind="ExternalInput")
    flowT = nc.dram_tensor("flowT", [16, 2, tcols], mybir.dt.float16,
                           kind="ExternalInput")
    flowO = nc.dram_tensor("flowO", [P, 2, ncols], mybir.dt.float16,
                           kind="ExternalInput")
    consts = nc.dram_tensor("consts", [P, 2], mybir.dt.float32,
                            kind="ExternalInput")
    out = nc.dram_tensor("out", [nchunk, 8, P, C], mybir.dt.int8,
                         kind="ExternalOutput")

    with TileContext(nc) as tc:
        with (
            tc.tile_pool(name="setup", bufs=1) as spool,
            tc.tile_pool(name="dram", bufs=1, space="DRAM") as dpool,
            tc.tile_pool(name="drs", bufs=2, space="DRAM") as dspool,
            tc.tile_pool(name="quad", bufs=1) as qpool,
            tc.tile_pool(name="main", bufs=2) as tp,
        ):
            quad = dpool.tile([QR, W, CW], mybir.dt.float32, tag="quad")
            quad_m = quad.rearrange("r w c -> (r w) c")

            pats = nc.inline_tensor(_patterns(), name="pats")
            pt = spool.tile([P, 1450], mybir.dt.float32, tag="pats")
            nc.sync.dma_start(out=pt[:], in_=pats[:, :])
            ct = spool.tile([P, 2], mybir.dt.float32, tag="consts")
            nc.sync.dma_start(out=ct[:], in_=consts[:, :])
            rowpat = pt[0:16, 0:640]
            colpat = pt[0:16, 640:1280]
            pat10 = pt[0:16, 1280:1290].rearrange("p (a b) -> p a b", b=1)
            rowO = pt[:, 1290:1370]
            colO = pt[:, 1370:1450]
            c0 = ct[0:16, 0:1]         # h*half
            c1 = ct[0:16, 1:2]         # h*half - m
            c0f = ct[:, 0:1]
            c1f = ct[:, 1:2]

            if VARIANT == "floor":
                # transfer-floor ablation: write the output once, no warp
                zt = spool.tile([P, G, 8, C], mybir.dt.int8, tag="zt")
                nc.vector.memset(zt[:], 0.0)
                for sg in range(nsg):
                    ov = out[sg * G:(sg + 1) * G].rearrange(
                        "j s p c -> p j s c")
                    nc.sync.dma_start(out=ov, in_=zt[:])
                nsg_run = 0
                qr_run = 0
            else:
                nsg_run = nsg
                qr_run = QR

            # ---- phase Q: build the quad table in DRAM ----
            for r0 in range(0, qr_run, P):
                nr = min(P, QR - r0)
                for c0q in range(0, W, QCW):
                    cwe = min(QCW, (W - 1) - c0q)      # valid quad cols
                    rw = min(QCW + 1, W - c0q)         # img cols to read
                    a = qpool.tile([P, QCW + 1, C], mybir.dt.int8,
                                   tag="qa")
                    nc.sync.dma_start(out=a[0:nr, 0:rw],
                                      in_=img[r0:r0 + nr, c0q:c0q + rw, :])
                    bt = qpool.tile([P, QCW + 1, C], mybir.dt.int8,
                                    tag="qb")
                    nc.sync.dma_start(out=bt[0:nr, 0:rw],
                                      in_=img[r0 + 1:r0 + 1 + nr,
                                              c0q:c0q + rw, :])
                    q = qpool.tile([P, QCW, CW], mybir.dt.float32, tag="qq")
                    nc.vector.tensor_copy(out=q[0:nr, 0:cwe, 0:16],
                                          in_=a[0:nr, 0:cwe])
                    nc.vector.tensor_copy(out=q[0:nr, 0:cwe, 16:32],
                                          in_=a[0:nr, 1:cwe + 1])
                    nc.vector.tensor_copy(out=q[0:nr, 0:cwe, 32:48],
                                          in_=bt[0:nr, 0:cwe])
                    nc.vector.tensor_copy(out=q[0:nr, 0:cwe, 48:64],
                                          in_=bt[0:nr, 1:cwe + 1])
                    cww = cwe
                    if cwe < QCW:
                        # fill the (never-indexed) last quad column with
                        # duplicated edge pixels so it is finite
                        for lo_, src in ((0, a), (16, a), (32, bt), (48, bt)):
                            nc.vector.tensor_copy(
                                out=q[0:nr, cwe:cwe + 1, lo_:lo_ + 16],
                                in_=src[0:nr, cwe:cwe + 1])
                        cww = cwe + 1
                    nc.sync.dma_start(out=quad[r0:r0 + nr, c0q:c0q + cww, :],
                                      in_=q[0:nr, 0:cww, :])

            tc.strict_bb_all_engine_barrier()

            # ---- phase M: indices, gather, bilinear ----
            for sg in range(nsg_run):
                sgf = float(8 * sg)
                ft = tp.tile([16, 2, 640], mybir.dt.float16, tag="ft")
                nc.sync.dma_start(
                    out=ft[:], in_=flowT[:, :, sg * 640:(sg + 1) * 640])
                ff = tp.tile([16, 2, 640], mybir.dt.float32, tag="ff")
                nc.vector.tensor_copy(out=ff[:], in_=ft[:])
                ffy = ff[:, 0, :]
                ffx = ff[:, 1, :]

                t1 = tp.tile([16, 640], mybir.dt.float32, tag="t1")
                t3 = tp.tile([16, 640], mybir.dt.float32, tag="t3")
                t4 = tp.tile([16, 640], mybir.dt.float32, tag="t4")
                t5 = tp.tile([16, 640], mybir.dt.float32, tag="t5")
                t6 = tp.tile([16, 640], mybir.dt.float32, tag="t6")
                t2i = tp.tile([16, 640], mybir.dt.int32, tag="t2i")
                loc16 = tp.tile([16, 640], mybir.dt.int16, tag="loc16")

                # qy = (rowpat + 8sg + h*half) - flowy ; clip to [0, Hf-1]
                nc.vector.tensor_scalar(out=t1[:], in0=rowpat, scalar1=c0,
                                        scalar2=sgf, op0=AOT.add, op1=AOT.add)
                nc.vector.tensor_tensor(out=t1[:], in0=t1[:], in1=ffy,
                                        op=AOT.subtract)
                nc.vector.tensor_scalar(out=t1[:], in0=t1[:],
                                        scalar1=float(Hf - 1), scalar2=0.0,
                                        op0=AOT.min, op1=AOT.max)
                # fy = min(floor(qyc), Hf-2)
                nc.vector.tensor_copy(out=t2i[:], in_=t1[:])
                nc.vector.tensor_copy(out=t3[:], in_=t2i[:])
                nc.vector.tensor_tensor(out=t4[:], in0=t3[:], in1=t1[:],
                                        op=AOT.is_gt)
                nc.vector.tensor_tensor(out=t3[:], in0=t3[:], in1=t4[:],
                                        op=AOT.subtract)
                nc.vector.tensor_scalar(out=t3[:], in0=t3[:],
                                        scalar1=float(Hf - 2), scalar2=None,
                                        op0=AOT.min)

                # qx = colpat - flowx ; clip to [0, W-1]
                nc.vector.tensor_tensor(out=t1[:], in0=colpat, in1=ffx,
                                        op=AOT.subtract)
                nc.vector.tensor_scalar(out=t1[:], in0=t1[:],
                                        scalar1=float(W - 1), scalar2=0.0,
                                        op0=AOT.min, op1=AOT.max)
                nc.vector.tensor_copy(out=t2i[:], in_=t1[:])
                nc.vector.tensor_copy(out=t5[:], in_=t2i[:])
                nc.vector.tensor_tensor(out=t6[:], in0=t5[:], in1=t1[:],
                                        op=AOT.is_gt)
                nc.vector.tensor_tensor(out=t5[:], in0=t5[:], in1=t6[:],
                                        op=AOT.subtract)
                nc.vector.tensor_scalar(out=t5[:], in0=t5[:],
                                        scalar1=float(W - 2), scalar2=None,
                                        op0=AOT.min)

                # weights in the gather-output (wrap128) layout, computed
                # from the wrap128 flow shipment on [128, 80] tiles
                fo = tp.tile([P, 2, 80], mybir.dt.float16, tag="fo")
                nc.sync.dma_start(
                    out=fo[:], in_=flowO[:, :, sg * 80:(sg + 1) * 80])
                fof = tp.tile([P, 2, 80], mybir.dt.float32, tag="fof")
                nc.vector.tensor_copy(out=fof[:], in_=fo[:])
                o1 = tp.tile([P, 80], mybir.dt.float32, tag="o1")
                o2 = tp.tile([P, 80], mybir.dt.float32, tag="o2")
                o3 = tp.tile([P, 80], mybir.dt.float32, tag="o3")
                o2i = tp.tile([P, 80], mybir.dt.int32, tag="o2i")
                ayO = tp.tile([P, 80, 1], mybir.dt.float32, tag="ayO")
                axO = tp.tile([P, 80, 1], mybir.dt.float32, tag="axO")
                ayOv = ayO[:].rearrange("p s e -> p (s e)")
                axOv = axO[:].rearrange("p s e -> p (s e)")
                for (is_row, ffc, wt) in ((True, fof[:, 0, :], ayOv),
                                          (False, fof[:, 1, :], axOv)):
                    hi = float(Hf - 1) if is_row else float(W - 1)
                    if is_row:
                        nc.vector.tensor_scalar(
                            out=o1[:], in0=rowO, scalar1=c0f, scalar2=sgf,
                            op0=AOT.add, op1=AOT.add)
                        nc.vector.tensor_tensor(out=o1[:], in0=o1[:],
                                                in1=ffc, op=AOT.subtract)
                    else:
                        nc.vector.tensor_tensor(out=o1[:], in0=colO,
                                                in1=ffc, op=AOT.subtract)
                    nc.vector.tensor_scalar(out=o1[:], in0=o1[:],
                                            scalar1=hi, scalar2=0.0,
                                            op0=AOT.min, op1=AOT.max)
                    nc.vector.tensor_copy(out=o2i[:], in_=o1[:])
                    nc.vector.tensor_copy(out=o2[:], in_=o2i[:])
                    nc.vector.tensor_tensor(out=o3[:], in0=o2[:], in1=o1[:],
                                            op=AOT.is_gt)
                    nc.vector.tensor_tensor(out=o2[:], in0=o2[:], in1=o3[:],
                                            op=AOT.subtract)
                    nc.vector.tensor_scalar(out=o2[:], in0=o2[:],
                                            scalar1=hi - 1.0, scalar2=None,
                                            op0=AOT.min)
                    nc.vector.tensor_tensor(out=wt, in0=o1[:], in1=o2[:],
                                            op=AOT.subtract)

                # loc = (fy - (h*half - m) - i0(ck)) * W + fx  -> int16
                wb = tp.tile([16, G, 1], mybir.dt.float32, tag="wb")
                nc.vector.tensor_scalar(out=wb[:], in0=pat10, scalar1=c1,
                                        scalar2=sgf, op0=AOT.add, op1=AOT.add)
                fy3 = t3.rearrange("p (a b) -> p a b", b=64)
                wb_b, fy_b = bass.broadcast_tensor_aps(wb[:], fy3)
                nc.vector.tensor_tensor(out=fy3, in0=fy_b, in1=wb_b,
                                        op=AOT.subtract)
                nc.vector.tensor_scalar(out=t3[:], in0=t3[:],
                                        scalar1=float(W), scalar2=None,
                                        op0=AOT.mult)
                nc.vector.tensor_tensor(out=t3[:], in0=t3[:], in1=t5[:],
                                        op=AOT.add)
                nc.vector.tensor_copy(out=loc16[:], in_=t3[:])

                # replicate loc16 x8 across partition groups (DRAM round
                # trip: two copies into scratch, then one read per quadrant)
                locS = dspool.tile([32, 640], mybir.dt.int16, tag="locS")
                nc.sync.dma_start(out=locS[0:16], in_=loc16[:])
                nc.sync.dma_start(out=locS[16:32], in_=loc16[:])
                locT = tp.tile([P, 640], mybir.dt.int16, tag="locT")
                for qd in range(4):
                    nc.sync.dma_start(out=locT[32 * qd:32 * qd + 32],
                                      in_=locS[:, :])

                # gather + bilinear
                gt = tp.tile([P, G, 8, CW], mybir.dt.float32, tag="gt")
                for j in range(G if VARIANT != "nogather" else 0):
                    i0 = _i0(sg * G + j)
                    nc.gpsimd.dma_gather(
                        out_ap=gt[:, j],
                        in_ap=quad_m[i0 * W:(i0 + WIN) * W, :],
                        idxs_ap=locT[:, j * 64:(j + 1) * 64],
                        num_idxs=K, num_idxs_reg=K, elem_size=CW,
                    )
                gv = gt.rearrange("p j s c -> p (j s) c")
                dif = tp.tile([P, G * 8, 32], mybir.dt.float32, tag="dif")
                nc.vector.tensor_tensor(out=dif[:], in0=gv[:, :, 32:64],
                                        in1=gv[:, :, 0:32], op=AOT.subtract)
                ay_b, dif_b = bass.broadcast_tensor_aps(ayO[:], dif[:])
                nc.vector.tensor_tensor(out=dif[:], in0=dif_b, in1=ay_b,
                                        op=AOT.mult)
                nc.vector.tensor_tensor(out=dif[:], in0=dif[:],
                                        in1=gv[:, :, 0:32], op=AOT.add)
                hd = tp.tile([P, G * 8, 16], mybir.dt.float32, tag="hd")
                nc.vector.tensor_tensor(out=hd[:], in0=dif[:, :, 16:32],
                                        in1=dif[:, :, 0:16], op=AOT.subtract)
                ax_b, hd_b = bass.broadcast_tensor_aps(axO[:], hd[:])
                nc.vector.tensor_tensor(out=hd[:], in0=hd_b, in1=ax_b,
                                        op=AOT.mult)
                nc.vector.tensor_tensor(out=hd[:], in0=hd[:],
                                        in1=dif[:, :, 0:16], op=AOT.add)
                out8 = tp.tile([P, G, 8, C], mybir.dt.int8, tag="out8")
                nc.vector.tensor_copy(
                    out=out8.rearrange("p j s c -> p (j s) c"), in_=hd[:])

                ov = out[sg * G:(sg + 1) * G].rearrange(
                    "j s p c -> p j s c")
                nc.sync.dma_start(out=ov, in_=out8[:])

    nc.compile()
    _PROGRAM_CACHE[key] = nc
    return nc


def _patterns():
    """Host-precomputed index patterns (identical for every core)."""
    c = np.arange(640)
    rowpat = np.broadcast_to((c // 80).astype(f32), (P, 640))
    colpat = (16 * (c % 80))[None, :] + (np.arange(P) % 16)[:, None]
    pat10 = np.broadcast_to(((4 * np.arange(10)) // 5).astype(f32), (P, 10))
    pl = np.arange(80)[None, :] * P + np.arange(P)[:, None]   # local pixel
    rowO = pl // W
    colO = pl % W
    return np.concatenate(
        [rowpat, colpat.astype(f32), pat10, rowO.astype(f32),
         colO.astype(f32)], axis=1).astype(f32)


def kernel(image, flow, half=HALF):
    image = np.asarray(image)
    flow = np.asarray(flow, dtype=f32)
    Hf = 2 * half

    fmax = _POOL.submit(lambda: float(np.abs(flow).max()))
    imaxs = list(_POOL.map(lambda b: float(np.abs(image[b]).max()),
                           range(N)))
    imax = max(imaxs)
    m = int(np.ceil(fmax.result())) + 2
    m = max(m, 3)
    assert m <= 12, m
    R = half + 2 * m + 2
    s = f32(127.0 / imax)

    tcols = half * W // 16

    def _prep(core):
        b, h = core // 2, core % 2
        hH = h * half
        rows = np.clip(np.arange(hH - m, hH - m + R), 0, Hf - 1)
        imgs = np.rint(image[b][rows] * s).astype(np.int8)
        fl = flow[b, hH:hH + half].reshape(tcols, 16, 2)
        ft = np.ascontiguousarray(
            fl.transpose(1, 2, 0)).astype(np.float16)  # (16, 2, tcols)
        flo = flow[b, hH:hH + half].reshape(-1, P, 2)
        fo = np.ascontiguousarray(
            flo.transpose(1, 2, 0)).astype(np.float16)  # (P, 2, ncols)
        consts = np.broadcast_to(
            np.array([hH, hH - m], dtype=f32), (P, 2))
        return {"img": imgs, "flowT": ft, "flowO": fo,
                "consts": np.ascontiguousarray(consts)}

    in_maps = list(_POOL.map(_prep, range(8)))

    nc = _build_program(m, half)
    res = run_bass_kernel_spmd(nc, in_maps, core_ids=list(range(8)))

    full = np.empty((N, Hf, W, C), dtype=f32)
    for core in range(8):
        b, h = core // 2, core % 2
        o = res.results[core]["out"]                   # (nchunk, 8, P, C)
        np.multiply(o.reshape(half, W, C), f32(imax / 127.0),
                    out=full[b, h * half:(h + 1) * half], dtype=f32,
                    casting="unsafe")
    return full
